# revision 1
# baseline (speedup 1.0000x reference)
"""Barrel shifter right 64 (zero-fill), batch 2097152, on 8 NeuronCores.

Layout: row-major. Each SBUF work tile holds 4096 rows: partition p carries 32
consecutive rows (spans), each span padded to 96 bf16 elements (32 zero guard +
64 data). A stage's shift-by-sa is a free-dim offset view whose low lanes read
the zero guard. Per-row mux select bits come from S = packed shift amount,
expanded once per tile to an int16 [128, 32*64] tile; stage i's mask is a
single 4x-mode tensor_scalar bitwise_and, and the mux itself is
tensor_copy + copy_predicated (2x mode).
"""

import sys

if "/opt/trn_rl_repo" not in sys.path:
    sys.path.insert(0, "/opt/trn_rl_repo")

import numpy as np

B_TOTAL = 2097152
NBITS = 64
NCTRL = 6
NCORES = 8
R_FULL = B_TOTAL // NCORES  # 262144 rows per core

P = 128
SPANS = 32                  # rows per partition per tile
TILE_ROWS = P * SPANS       # 4096
PITCH = 96                  # guard(32) + bits(64)
GUARD = 32
FD = SPANS * NBITS          # 2048
SFD = SPANS * NCTRL         # 192

_built = {}


def build(rows, repeat=1):
    import concourse.bass as bass
    from concourse import mybir

    f32 = mybir.dt.float32
    bf16 = mybir.dt.bfloat16
    i16 = mybir.dt.int16
    Alu = mybir.AluOpType

    nt_data = rows // TILE_ROWS
    nt = nt_data * repeat
    assert rows % TILE_ROWS == 0

    nc = bass.Bass()
    data = nc.declare_dram_parameter("data", [rows, NBITS], f32, isOutput=False)
    shift = nc.declare_dram_parameter("shift", [rows, NCTRL], f32, isOutput=False)
    out = nc.declare_dram_parameter("out", [rows, NBITS], f32, isOutput=True)

    data_r = data.rearrange("(n p t) k -> n p (t k)", p=P, t=SPANS)
    shift_r = shift.rearrange("(n p t) k -> n p (t k)", p=P, t=SPANS)
    out_r = out.rearrange("(n p t) k -> n p (t k)", p=P, t=SPANS)

    def sb(name, shape, dt):
        return nc.alloc_sbuf_tensor(name, shape, dt)

    dtile = [sb(f"dtile{j}", [P, FD], f32) for j in (0, 1)]
    stile = [sb(f"stile{j}", [P, SFD], f32) for j in (0, 1)]
    wts = sb("wts", [P, NCTRL], f32)
    wsum = [sb(f"wsum{j}", [P, SFD], f32) for j in (0, 1)]
    Sf = [sb(f"Sf{j}", [P, SPANS], f32) for j in (0, 1)]
    Sx = [sb(f"Sx{j}", [P, FD], i16) for j in (0, 1)]
    msk = [sb(f"msk{j}", [P, FD], i16) for j in (0, 1)]
    wk = [sb(f"wk{j}", [P, SPANS * PITCH], bf16) for j in range(4)]
    otile = [sb(f"otile{j}", [P, FD], f32) for j in (0, 1)]

    def spans(t, off=GUARD):
        return t.ap().rearrange("p (t c) -> p t c", c=PITCH)[:, :, off:off + NBITS]

    with (
        nc.Block() as block,
        nc.semaphore("s_din0") as s_din0,
        nc.semaphore("s_din1") as s_din1,
        nc.semaphore("s_dout0") as s_dout0,
        nc.semaphore("s_dout1") as s_dout1,
        nc.semaphore("s_dve") as s_dve,
    ):
        # DMA-completion semaphores are per buffer slot: queue completions are
        # FIFO per queue, not globally, so a single accumulated count can be
        # reached by a later tile's fast DMA while an earlier tile's small DMA
        # still has descriptors in flight. Within one slot, issue order is
        # gated by compute completion, so per-slot counts are well ordered.
        s_din = [s_din0, s_din1]
        s_dout = [s_dout0, s_dout1]

        @block.sync
        def _(sp):
            for n in range(nt):
                c = n & 1
                if n >= 2:
                    sp.wait_ge(s_dve, n - 1)
                sp.dma_start(
                    out=dtile[c].ap(), in_=data_r[n % nt_data]
                ).then_inc(s_din[c], 16)
                sp.dma_start(
                    out=stile[c].ap(), in_=shift_r[n % nt_data]
                ).then_inc(s_din[c], 16)
                if n >= 2:
                    sp.dma_start(
                        out=out_r[(n - 2) % nt_data], in_=otile[c].ap()
                    ).then_inc(s_dout[c], 16)
            for n in (nt - 2, nt - 1):
                c = n & 1
                sp.wait_ge(s_dve, n + 1)
                sp.dma_start(
                    out=out_r[n % nt_data], in_=otile[c].ap()
                ).then_inc(s_dout[c], 16)
            sp.wait_ge(s_dout0, 16 * ((nt + 1) // 2))
            sp.wait_ge(s_dout1, 16 * (nt // 2))

        @block.vector
        def _(v):
            # DVE ops shorter than the drain threshold (~266ns) do not flush
            # their SBUF writes before the next instruction issues, so a
            # consumer op that follows a short producer reads torn data. The
            # wts memsets (free size 1) run first; the four big guard-zero
            # AND-0 ops after them provide microseconds of write-flush
            # spacing before wts is first read.
            for j in range(NCTRL):
                v.memset(wts.ap()[:, j:j + 1], float(1 << (NCTRL - 1 - j)))
            for j in range(4):
                g = wk[j].ap().bitcast(i16)
                v.tensor_scalar(g, g, 0, None, Alu.bitwise_and)
            for n in range(nt):
                c = n & 1
                v.wait_ge(s_din[c], 32 * (n // 2 + 1))
                if n >= 2:
                    v.wait_ge(s_dout[c], 16 * (n // 2))
                A, Bw = wk[2 * c], wk[2 * c + 1]

                # S = sum_j 2^(5-j) * shift[:, j], per row. Each short op in
                # this chain is followed by a long unrelated op so its writes
                # flush before the dependent read (short-op RAW hazard).
                st3 = stile[c].ap().rearrange("p (t j) -> p t j", j=NCTRL)
                ws3 = wsum[c].ap().rearrange("p (t j) -> p t j", j=NCTRL)
                w3 = wts.ap().unsqueeze(1).broadcast_to([P, SPANS, NCTRL])
                v.tensor_tensor(ws3, st3, w3, Alu.mult)
                # convert incoming rows to bf16 work layout (spacer for wsum)
                d3 = dtile[c].ap().rearrange("p (t k) -> p t k", k=NBITS)
                v.tensor_copy(spans(A), d3)
                v.tensor_reduce(
                    Sf[c].ap().unsqueeze(2), ws3, mybir.AxisListType.X, Alu.add
                )
                # spacer between the short reduce and the Sf-reading expansion
                v.tensor_scalar(msk[c].ap(), msk[c].ap(), 0, None, Alu.bitwise_and)
                # expand S to per-bit-lane int16
                v.tensor_copy(
                    Sx[c].ap().rearrange("p (t k) -> p t k", k=NBITS),
                    Sf[c].ap().unsqueeze(2).broadcast_to([P, SPANS, NBITS]),
                )

                def strip(t, lo, hi, off=0):
                    # [128, SPANS, hi-lo] view at span-local elems [lo+off, hi+off)
                    return t.ap().rearrange("p (t c) -> p t c", c=PITCH)[
                        :, :, GUARD + lo + off:GUARD + hi + off
                    ]

                def mstrip(lo, hi):
                    return msk[c].ap().rearrange("p (t k) -> p t k", k=NBITS)[
                        :, :, lo:hi
                    ]

                m3 = mstrip(0, NBITS)
                # stage 0 (sa=1): odd shift misaligns the predicated read; do
                # the shift in a copy A->B and predicate unshifted A over it
                # with the inverted mask.
                v.tensor_scalar(
                    msk[c].ap(), Sx[c].ap(), 1, 1,
                    Alu.bitwise_and, Alu.bitwise_xor,
                )
                v.tensor_copy(spans(Bw), spans(A, GUARD - 1))
                v.copy_predicated(spans(Bw), m3, spans(A))
                # stages 1..5 run in place on B. For sa <= 8 the shifted read
                # trails the write cursor by less than the pipeline depth, so
                # it observes pre-op data. For sa = 16/32, split the span into
                # sa-wide strips processed high to low: each strip reads only
                # regions no strip has written.
                for i in range(1, 6):
                    sa = 1 << i
                    v.tensor_scalar(
                        msk[c].ap(), Sx[c].ap(), 1 << i, None, Alu.bitwise_and
                    )
                    if sa <= 8:
                        v.copy_predicated(spans(Bw), m3, spans(Bw, GUARD - sa))
                    else:
                        for lo in range(NBITS - sa, -sa, -sa):
                            lo = max(lo, 0)
                            v.copy_predicated(
                                strip(Bw, lo, lo + sa),
                                mstrip(lo, lo + sa),
                                strip(Bw, lo, lo + sa, -sa),
                            )

                o3 = otile[c].ap().rearrange("p (t k) -> p t k", k=NBITS)
                v.tensor_copy(o3, spans(Bw)).then_inc(s_dve, 1)

    return nc


def _get(rows, repeat=1):
    key = (rows, repeat)
    if key not in _built:
        _built[key] = build(rows, repeat)
    return _built[key]


def run_cores(data, shift, rows, trace=False):
    from concourse.bass_utils import run_bass_kernel_spmd

    nc = _get(rows)
    ncores = data.shape[0] // rows
    in_maps = [
        {
            "data": np.ascontiguousarray(data[i * rows:(i + 1) * rows]),
            "shift": np.ascontiguousarray(shift[i * rows:(i + 1) * rows]),
        }
        for i in range(ncores)
    ]
    res = run_bass_kernel_spmd(nc, in_maps, list(range(ncores)), trace=trace)
    full = np.concatenate([res.results[i]["out"] for i in range(ncores)], axis=0)
    return full, res


def kernel(data, shift):
    data = np.ascontiguousarray(np.asarray(data), dtype=np.float32)
    shift = np.ascontiguousarray(np.asarray(shift), dtype=np.float32)
    full, _ = run_cores(data, shift, R_FULL)
    return full.astype(np.float32, copy=False)

